# revision 53
# baseline (speedup 1.0000x reference)
"""Trainium2 Bass kernel for nn_MultiHeadAttention_38611755991513.

Reference computation (B=2, D=1024, L=2048, H=16, DK=64):
    q/k/v = conv1d(kernel=1) projections of query [B, D, L]
    att   = softmax(mask(q^T k / sqrt(DK)))   with key-only mask [B, 1, L]
    out   = Wo @ (att @ v heads recombined) + bo

Sharding: 32 (batch, head) pairs -> 4 heads (one batch) per core.
Each core computes its 4 heads' attention plus the partial O-projection
(Wo columns for its heads); the host sums the 4 partials per batch.

Key optimizations:
- Key-only mask -> masked keys compacted away on the host; the ragged last
  128-key tile overlaps the previous one so every tile is full width (the
  duplicated rows are dead: zeroed V rows and ones-column).
- Scores are computed transposed (S^T[k, q]) so exp(S^T) is directly the
  moving operand of att@v; the softmax denominator comes free as a 65th
  "ones" column of the V operand.
- Per (head-pair, key-tile) the two heads' scores land in ONE [128, 2, 512]
  PSUM tile (the K=64 matmuls target row groups 0-63 / 64-127 and stream
  concurrently when issued back-to-back) and are consumed by ONE merged
  exp, which frees both heads' slots simultaneously.
- All DRAM parameters are host-swizzled so both the DRAM source AND the
  SBUF destination of every input DMA are contiguous per-partition runs
  (wq/wk head-pair-major, xb 256-column-chunk-major with 3D moving APs,
  out query-chunk-major): the DMA element size is min(src_run, dst_run),
  and strided destinations were a hidden 256-byte-element bottleneck.
- All input DMAs stay on the single Sync queue in consumption order (bulk
  last): the Tile scheduler models DMA serially per queue, so one queue
  keeps its ETAs accurate and its static schedule matched to reality.
- Phase 0 uses precise per-tile filler deadlines (each k chunk emitted just
  before the first score tile reading it, v chains after their tile's
  scores) plus tile_wait_until hints on late-xk chains, so the early score/
  exp stream is never queued behind fillers stalled on in-flight DMAs and
  the HAM clock stays at 2.4 GHz from ~17us on.
- Tail: the final phase's att@v runs as two 256-wide halves accumulating in
  disjoint column regions of ONE psum tile per head (second half starts
  without re-clearing the bank, so no WAR wait on the first half's
  normalize); the first half's normalize + O chunks overlap the second
  half's matmuls; the last normalizes use a PE broadcast; final O-chunk
  copies alternate Vector/Scalar and rotate through 4 PSUM banks; the
  output flushes are split into quarters so the last DMA is small.
- bv is folded into bo on the host (out = Wo@(y/den) + (Wo@bv + bo)), and
  bq is pre-scaled by 1/sqrt(DK).
"""

import sys

sys.path.insert(0, "/opt/trn_rl_repo")

import numpy as np
import ml_dtypes

import concourse.bass as bass
import concourse.tile as tile
from concourse import bacc, mybir
from concourse.bass_utils import run_bass_kernel_spmd

B, D, L, H = 2, 1024, 2048, 16
DK = 64
NCORES = 8
HPC = 4              # heads per core
DH = HPC * DK        # 256 head-dims per core
KT = D // 128        # 8 contraction tiles for the projections
BF16 = mybir.dt.bfloat16
F32 = mybir.dt.float32
NPBF16 = ml_dtypes.bfloat16

TRACE = False            # set True (e.g. from test.py) to capture a HW profile
LAST_EXEC_NS = None
LAST_RESULTS = None

QBW = 512                # query-block width (one PSUM bank per head slot)
NQB = L // QBW


def _chunks(total, size):
    out = []
    s = 0
    while s < total:
        w = min(size, total - s)
        out.append((s, w))
        s += w
    return out


def _key_tiles(L_c):
    """Full-width 128-key tiles covering [0, L_c); the last tile overlaps the
    previous one when L_c is ragged (its first MT*128-L_c rows are dead)."""
    MT = (L_c + 127) // 128
    mts = [(i * 128, 128) for i in range(MT - 1)]
    mts.append((L_c - 128, 128))
    return mts


def _xk_blocks(L_c):
    """Column blocks of the compacted keys, matching k-chain consumption."""
    if L_c <= 128:
        return [(0, L_c)]
    if L_c <= 512:
        return [(0, 128), (128, L_c - 128)]
    if L_c <= 768:
        return [(0, 128), (128, 384), (512, L_c - 512)]
    return [(0, 128), (128, 384), (512, 256), (768, L_c - 768)]


def _build(L_c):
    """Build + compile the per-core Bass program for compacted key length L_c."""
    assert L_c >= 128
    nc = bacc.Bacc("TRN2", debug=False, num_devices=NCORES)
    mts = _key_tiles(L_c)
    MT = len(mts)
    deadw = MT * 128 - L_c   # dead leading rows of the (overlapped) tail tile
    EXP = mybir.ActivationFunctionType.Exp
    xkb = _xk_blocks(L_c)

    xb_d = nc.declare_dram_parameter("xb", [128, 2 * NQB, KT, 256], BF16, isOutput=False)
    xk_ds = [
        nc.declare_dram_parameter(f"xk{i}", [128, KT, w], BF16, isOutput=False)
        for i, (s, w) in enumerate(xkb)
    ]
    vs_d = nc.declare_dram_parameter("vsetup", [128, MT, HPC, 65], BF16, isOutput=False)
    wq_d = nc.declare_dram_parameter("wq", [128, 2, KT, 128], BF16, isOutput=False)
    wk_d = nc.declare_dram_parameter("wk", [128, 2, KT, 128], BF16, isOutput=False)
    wv_d = nc.declare_dram_parameter("wv", [128, KT, DH], BF16, isOutput=False)
    wo_d = nc.declare_dram_parameter("wo", [128, 2, D], BF16, isOutput=False)
    bias_d = nc.declare_dram_parameter("bias", [128, 4], F32, isOutput=False)
    out_d = nc.declare_dram_parameter("out", [NQB, 128, KT, QBW], BF16, isOutput=True)

    from contextlib import ExitStack
    with tile.TileContext(nc) as tc, ExitStack() as ctx:
        pers = ctx.enter_context(tc.tile_pool(name="pers", bufs=1))

        def ptile(shape, dtype, name):
            return pers.tile(shape, dtype, tag=name, name=name)

        # persistent SBUF tensors.  wq/wk are head-pair-major and xb is
        # 256-column-chunk-major so each input DMA writes one fully
        # CONTIGUOUS per-partition run (the element size a DMA moves is
        # min(src_run, dst_run) -- strided SBUF destinations were the
        # hidden 256-byte-element bottleneck).
        xk_a = ptile([128, KT, L_c], BF16, "xk_a")
        xb_a = ptile([128, 2 * NQB, KT, 256], BF16, "xb_a")
        wq_a = ptile([128, 2, KT, 128], BF16, "wq_a")
        wk_a = ptile([128, 2, KT, 128], BF16, "wk_a")
        wv_a = ptile([128, KT, DH], BF16, "wv_a")
        wo_a = ptile([128, 2, D], BF16, "wo_a")
        xk_t = [xk_a[:, i] for i in range(KT)]
        wv_t = [wv_a[:, i] for i in range(KT)]
        wo_t = [wo_a[:, i] for i in range(2)]

        def xb_sl(kk, qs, jw):
            """Moving-operand view of xb columns [qs, qs+jw) for D-tile kk."""
            c0, nch = qs // 256, jw // 256
            if nch == 1:
                return xb_a[:, c0, kk]
            return xb_a[:, c0:c0 + nch, kk]   # [128, nch, 256] free=jw
        bias_all = ptile([128, 4], F32, "bias_all")
        bq_t = [bias_all[:, 2 * i + 0:2 * i + 1] for i in range(2)]
        bk_t = [bias_all[:, 2 * i + 1:2 * i + 2] for i in range(2)]
        q_t = [ptile([128, L], BF16, f"q{i}") for i in range(2)]
        k_t = [ptile([128, L_c], BF16, f"k{i}") for i in range(2)]
        z_t = [ptile([128, L], BF16, f"z{i}") for i in range(2)]
        # V operand per key tile: [128, head, 65]; col 64 is the ones column
        # (denominator); vsetup pre-zeroes dead rows and sets the ones
        vs_a = ptile([128, MT, HPC, 65], BF16, "vs_a")
        va_t = [vs_a[:, mt] for mt in range(MT)]
        # per-qblock output staging, so each qblock stores with ONE DMA
        ob_a = [ptile([128, 8, QBW], BF16, f"ob{i}") for i in range(2)]
        ones_t = ptile([1, 64], F32, "ones_t")
        # ---- input DMAs: three queues (sync/scalar HWDGE + gpsimd SWDGE)
        # issue in parallel, each in consumption order, with the bulk last:
        # in-flight DMAs share the ~360 GB/s fabric at packet granularity,
        # so anything enqueued early steals bandwidth from the critical
        # first-wave slices.
        # NOTE: scalar (ACT) also has an HWDGE ring but issuing DMAs there
        # delays the activation-table load and the first exp -- keep it clean.
        # single queue by default: the Tile scheduler models DMA transfers
        # serially per queue, and with one queue its ETAs match reality --
        # split queues made it schedule consumers ahead of slow transfers.
        import os
        if os.environ.get("MULTI_QUEUE"):
            q_xk, q_rest = nc.gpsimd, nc.sync
        else:
            q_xk = q_rest = nc.sync
        # critical wave: first k chain needs xk[:,0:128]+wk half 0; the
        # prologue q chains need wq half 0 + xb cols 0:512; biases gate the
        # first PSUM copy-outs.
        # sync carries only the small critical set; the xk blocks AND the
        # late-consumed bulk (xb blocks 2-7, wo) queue serially on gpsimd so
        # the bulk cannot steal fabric bandwidth from xk during the ramp
        # (phase 0 consumes xk progressively and is transfer-bound).
        q_xk.dma_start(xk_a[:, :, xkb[0][0]:xkb[0][0] + xkb[0][1]], xk_ds[0][:])
        q_rest.dma_start(wk_a[:, 0], wk_d[:, 0])
        q_rest.dma_start(bias_all[:], bias_d[:])
        q_rest.dma_start(wq_a[:, 0], wq_d[:, 0])
        q_rest.dma_start(xb_a[:, 0:2], xb_d[:, 0:2])
        for i in range(1, len(xkb)):
            s, w = xkb[i]
            q_xk.dma_start(xk_a[:, :, s:s + w], xk_ds[i][:])
        q_rest.dma_start(wk_a[:, 1], wk_d[:, 1])
        q_rest.dma_start(wv_a[:], wv_d[:])
        q_rest.dma_start(wq_a[:, 1], wq_d[:, 1])
        q_xk.dma_start(vs_a[:], vs_d[:])
        q_xk.dma_start(xb_a[:, 2:4], xb_d[:, 2:4])
        q_xk.dma_start(xb_a[:, 4:6], xb_d[:, 4:6])
        q_xk.dma_start(wo_a[:], wo_d[:])
        q_xk.dma_start(xb_a[:, 6:8], xb_d[:, 6:8])

        with (
            tc.tile_pool(name="psA", bufs=2, space="PSUM") as pa,
            tc.tile_pool(name="psY", bufs=2, space="PSUM") as pb,
            tc.tile_pool(name="psO", bufs=2, space="PSUM") as pox,
            tc.tile_pool(name="pexp", bufs=2 * MT + 4) as pp,
            tc.tile_pool(name="small", bufs=3) as psm,
        ):
            def k_chain(kt, ns, nw, halves=None, wait=None):
                kp = pox.tile([128, nw], F32, tag="po", name=f"kp{kt}_{ns}")

                def half(lo, hi):
                    # `wait` floors the Tile scheduler's assumed ready time:
                    # its DMA model ignores fabric contention, so without it
                    # late-xk chains get scheduled AHEAD of already-ready
                    # scores and the PE stalls on the real transfer.
                    from contextlib import nullcontext
                    with tc.tile_wait_until(wait) if wait else nullcontext():
                        for kk in range(lo, hi):
                            nc.tensor.matmul(
                                kp[:],
                                wk_a[:, kt, kk],
                                xk_t[kk][:, ns:ns + nw],
                                start=(kk == 0), stop=(kk == KT - 1),
                            )
                        if hi == KT:
                            nc.vector.tensor_scalar_add(
                                k_t[kt][:, ns:ns + nw], kp[:], bk_t[kt][:]
                            )
                if halves is None:
                    half(0, KT)
                else:
                    halves.append(lambda: half(0, KT // 2))
                    halves.append(lambda: half(KT // 2, KT))

            def q_chain(qs, kt, halves=None, jw=QBW):
                qp = pox.tile([128, jw], F32, tag="po", name=f"qp{kt}_{qs}")

                def half(lo, hi):
                    for kk in range(lo, hi):
                        nc.tensor.matmul(
                            qp[:],
                            wq_a[:, kt, kk],
                            xb_sl(kk, qs, jw),
                            start=(kk == 0), stop=(kk == KT - 1),
                        )
                    if hi == KT:
                        nc.vector.tensor_scalar_add(q_t[kt][:, qs:qs + jw], qp[:], bq_t[kt][:])
                if halves is None:
                    half(0, KT)
                else:
                    halves.append(lambda: half(0, KT // 2))
                    halves.append(lambda: half(KT // 2, KT))

            def v_chain(mt):
                from contextlib import nullcontext
                ms, mw = mts[mt]
                wait = (0.021 + 0.0012 * (mt - 4)) if ms + mw > 512 else None
                vp = pox.tile([mw, DH], F32, tag="po", name=f"vp{mt}")
                with tc.tile_wait_until(wait) if wait else nullcontext():
                    for kk in range(KT):
                        nc.tensor.matmul(
                            vp[:],
                            xk_t[kk][:, ms:ms + mw],
                            wv_t[kk][:],
                            start=(kk == 0), stop=(kk == KT - 1),
                        )
                    for h in range(HPC):
                        nc.vector.tensor_copy(
                            va_t[mt][:, h, 0:64], vp[:, h * 64:(h + 1) * 64]
                        )
                    if mt == MT - 1 and deadw:
                        # re-zero the dead overlap rows the copy just filled
                        nc.vector.memset(va_t[mt][0:deadw, :, 0:64], 0)

            def o_chunk(qi, qs, m8, ow=QBW, oo=0, eng=None, alt=False):
                # alt: borrow the y pool's (free-at-drain) PSUM ring so the
                # final chunks rotate through 4 banks instead of 2 -- the
                # drain is paced by the copy-out round-trip latency.
                if alt:
                    op = pb.tile([128, ow], F32, tag="y", name=f"o{qs}_{m8}_{oo}")
                else:
                    op = pox.tile([128, ow], F32, tag="po", name=f"o{qs}_{m8}_{oo}")
                for kt in range(2):
                    nc.tensor.matmul(
                        op[:],
                        wo_t[kt][:, m8 * 128:(m8 + 1) * 128],
                        z_t[kt][:, qs + oo:qs + oo + ow],
                        start=(kt == 0), stop=(kt == 1),
                    )
                dst = ob_a[qi % 2][:, m8, oo:oo + ow]
                if eng == "scalar":
                    nc.scalar.copy(dst, op[:])
                else:
                    nc.vector.tensor_copy(dst, op[:])

            def o_flush(qi, qs, lo=0, hi=8, co=0, cw=QBW):
                nc.sync.dma_start(
                    out_d[qi][:, lo:hi, co:co + cw],
                    ob_a[qi % 2][:, lo:hi, co:co + cw],
                )

            # ---- prologue ----
            # DMA-independent warmup on the zeroed dummy tiles: keeps PE busy
            # from ~7us (framework preamble end) so HAM un-throttles by the
            # time real data lands, without delaying the first k chain much.
            if L_c <= 128:
                kchunks = [(0, L_c)]
            elif L_c <= 512:
                kchunks = [(0, 128), (128, L_c - 128)]
            else:
                kchunks = [(0, 128), (128, 384)] + _chunks(L_c, 512)[1:]
            nc.vector.memset(ones_t[:], 1.0)
            k_chain(0, *kchunks[0])
            # HAM warmup on already-loaded data: keeps the PE streaming while
            # the first Q-block DMAs land so the clock ramps to 2.4 GHz
            for w in range(7):
                wu = pox.tile([128, 128], F32, tag="po", name=f"wu{w}")
                for kk in range(KT):
                    nc.tensor.matmul(
                        wu[:],
                        wk_a[:, 0, kk],
                        xk_t[kk][:, 0:128],
                        start=(kk == 0), stop=(kk == KT - 1),
                    )
            q_chain(0, 0, jw=256)
            q_chain(256, 0, jw=256)

            # ---- software-pipelined attention, head-PAIR phases ----
            def y_head(h, qs, p_tiles, yq, yw=QBW, yo=0, pe_bcast=False, share=None,
                       ypool=None):
                # ypool: the drain's second halves allocate from the score
                # pool's (free-by-then) slots instead of rotating pb's two --
                # Tile tracks PSUM deps per TILE, so any sharing or rotation
                # falsely serializes the halves against the normalize reads.
                state = {} if share is None else share

                def y_mt(mt):
                    first_owner = "yp" not in state
                    if mt == 0 and first_owner:
                        if ypool is not None:
                            state["yp"] = ypool.tile([65, QBW], F32, tag="wide",
                                                     name=f"y{qs}_{h}_{yo}")
                        else:
                            state["yp"] = pb.tile([65, QBW], F32, tag="y",
                                                  name=f"y{qs}_{h}_{yo}")
                    # a second half sharing the tile must NOT re-clear the
                    # bank (start=True zeroes the whole bank's has_written):
                    # its region is untouched, so plain accumulate-mode
                    # writes it fresh via the per-element has_written bits.
                    nc.tensor.matmul(
                        state["yp"][:, yo:yo + yw],
                        va_t[mt][:, h, :],
                        p_tiles[mt][:, h % 2, yo:yo + yw],
                        start=(mt == 0 and (share is None or first_owner)),
                        stop=(mt == MT - 1),
                        skip_group_check=(share is not None),
                    )

                def finish():
                    pt, off = h // 2, (h % 2) * 64
                    yp = state["yp"]
                    rt = psm.tile([1, QBW], F32, tag="rrow", name=f"rt{qs}_{h}_{yo}")
                    nc.vector.tensor_copy(rt[:, 0:yw], yp[64:65, yo:yo + yw])
                    rc = psm.tile([1, QBW], F32, tag="recip", name=f"rc{qs}_{h}_{yo}")
                    nc.vector.reciprocal_approx_fast(rc[:, 0:yw], rt[:, 0:yw])
                    zsl = z_t[pt][off:off + 64, qs + yo:qs + yo + yw]
                    if pe_bcast:
                        rbp = pox.tile([128, QBW], F32, tag="po", name=f"rb{qs}_{h}_{yo}")
                        nc.tensor.matmul(
                            rbp[0:64, 0:yw], ones_t[:], rc[:, 0:yw],
                            start=True, stop=True,
                        )
                        rbs = psm.tile([64, QBW], F32, tag="rb", name=f"rs{qs}_{h}_{yo}")
                        nc.vector.tensor_copy(rbs[:, 0:yw], rbp[0:64, 0:yw])
                        nc.vector.tensor_mul(zsl, yp[0:64, yo:yo + yw], rbs[:, 0:yw])
                    else:
                        rb = psm.tile([64, QBW], F32, tag="rb", name=f"rb{qs}_{h}_{yo}")
                        nc.gpsimd.partition_broadcast(rb[:, 0:yw], rc[:, 0:yw])
                        nc.vector.tensor_mul(zsl, yp[0:64, yo:yo + yw], rb[:, 0:yw])

                for mt in range(0, MT, 2):
                    def two(mt=mt):
                        y_mt(mt)
                        if mt + 1 < MT:
                            y_mt(mt + 1)
                    yq.append(two)
                yq.append(finish)
                return finish

            fillers = []   # (cost, emit) pairs
            fi = 0

            def pop_fillers(budget):
                nonlocal fi
                while budget > 0 and fi < len(fillers):
                    cost, emit = fillers[fi]
                    emit()
                    fi += 1
                    budget -= cost
                return budget

            def pop_until(idx):
                nonlocal fi
                while fi < idx:
                    fillers[fi][1]()
                    fi += 1

            # K/Q/V chains are PREREQUISITES of later emissions: Tile derives
            # dependencies from emission order, so a consumer emitted before
            # its writer would silently read stale data.
            def k_wait(ns):
                # approximate real arrival of the xk columns feeding [ns, ...)
                if ns < 384:
                    return None
                return 0.019 + 0.000009 * ns

            halves = []
            k0_dead_raw = []   # (first tile needing this chunk, filler idx after it)
            for ns, nw in kchunks[1:]:
                k_chain(0, ns, nw, halves, wait=k_wait(ns))
                t_first = next(t for t, (ms, mw) in enumerate(mts) if ms + mw > ns)
                k0_dead_raw.append((t_first, len(halves)))
            k0_end = len(halves)
            for ns, nw in kchunks:
                k_chain(1, ns, nw, halves, wait=k_wait(ns))
            fillers.extend((4, fn) for fn in halves)
            vk_deadline = len(fillers)
            # phase-0 per-tile deadlines: each k(0) chunk is emitted just
            # before the first tile whose scores read it (so early tiles'
            # scores aren't queued behind fillers waiting on late xk DMA),
            # and the k(1)/q(0,1) chains spread over the last 4 tiles.
            K0_DEAD = [0] * MT
            for t_first, idx in k0_dead_raw:
                for t in range(t_first, MT):
                    K0_DEAD[t] = max(K0_DEAD[t], idx)
            for j, t in enumerate(range(max(0, MT - 4), MT)):
                frac = (j + 1) / min(4, MT)
                tgt = k0_end + int(frac * (vk_deadline + 2 - k0_end))
                K0_DEAD[t] = max(K0_DEAD[t], tgt)
            deadline = {}
            after_block = {}
            for qi in range(NQB):
                for hp in range(2):
                    if (qi, hp) == (0, 0):
                        continue   # prologue chains
                    halves = []
                    q_chain(qi * QBW, hp, halves)
                    fillers.extend((4, fn) for fn in halves)
                after_block[qi] = len(fillers)
            # pop each block's q chains ONE PHASE EARLY: their bias copies
            # (DVE) then land before the phase that reads them starts, so the
            # phase's first score pair doesn't stall on the copy latency.
            for qi in range(NQB):
                nxt = min(qi + 1, NQB - 1)
                deadline[(qi, 0)] = deadline[(qi, 1)] = after_block[nxt]

            qblocks = _chunks(L, QBW)
            yq = []       # pending y work units of the previous pair

            for qi, (qs, qw) in enumerate(qblocks):
                for hp in range(2):
                    hA, hB = 2 * hp, 2 * hp + 1
                    first_phase = (qi, hp) == (0, 0)
                    last_phase = (qi, hp) == (len(qblocks) - 1, 1)
                    pop_until(deadline.get((qi, hp), 0))   # q chains this phase reads
                    if hp == 1 and qi >= 1:
                        # z of block qi-1 completed during the previous phase:
                        # its O-projection chunks become filler work now.
                        pqs = qblocks[qi - 1][0]
                        for m8 in range(8):
                            fillers.append(
                                (4, lambda qi=qi, pqs=pqs, m8=m8: o_chunk(qi - 1, pqs, m8))
                            )
                        fillers.append((0, lambda qi=qi, pqs=pqs: o_flush(qi - 1, pqs)))
                    ptiles = []
                    for mt, (ms, mw) in enumerate(mts):
                        for _ in range(2):
                            if yq:
                                yq.pop(0)()
                        if first_phase:
                            pop_until(min(K0_DEAD[mt], vk_deadline + 2))
                        else:
                            pop_fillers(4)
                        sp = pa.tile([128, 2, QBW], F32, tag="wide", name=f"s{qs}_{hp}_{mt}")
                        for sl, off in ((0, 0), (1, 64)):
                            nc.tensor.matmul(
                                sp[:, sl, :],
                                k_t[hp][off:off + 64, ms:ms + mw],
                                q_t[hp][off:off + 64, qs:qs + QBW],
                                start=True, stop=True,
                            )
                        px = pp.tile([128, 2, QBW], BF16, tag="p", name=f"p{qs}_{hp}_{mt}")
                        nc.scalar.activation(px[:], sp[:], EXP)
                        ptiles.append(px)
                        if first_phase:
                            # after the tile's scores: the v chain only feeds
                            # the y units at phase end, and wv/vs land late
                            v_chain(mt)
                    while yq:
                        yq.pop(0)()
                    yq = []
                    if first_phase:
                        pop_until(vk_deadline)   # v_chains feed the y units below
                    if not last_phase:
                        y_head(hA, qs, ptiles, yq)
                        y_head(hB, qs, ptiles, yq)
                    else:
                        last_ptiles = ptiles

            # ---- drain (last phase = (NQB-1, 1)) ----
            # The final pair's att@v runs in two 256-wide halves so the first
            # half's normalize + O chunks overlap the second half's matmuls.
            qi = len(qblocks) - 1
            qs = qblocks[-1][0]
            hA, hB = 2, 3
            HW2 = QBW // 2
            ylo = []
            finA_lo = y_head(hA, qs, last_ptiles, ylo, yw=HW2, yo=0)
            finB_lo = y_head(hB, qs, last_ptiles, ylo, yw=HW2, yo=0)
            # emit lo y matmuls now (finishes held), interleaving pairs
            for fn in ylo:
                if fn not in (finA_lo, finB_lo):
                    fn()
            yhi = []
            finA_hi = y_head(hA, qs, last_ptiles, yhi, yw=HW2, yo=HW2,
                             pe_bcast=True, ypool=pa)
            finB_hi = y_head(hB, qs, last_ptiles, yhi, yw=HW2, yo=HW2,
                             pe_bcast=True, ypool=pa)
            for fn in yhi:
                if fn not in (finA_hi, finB_hi):
                    fn()
            finA_lo()   # gpsimd-broadcast path, overlaps the hi matmuls
            finB_lo()
            finA_hi()
            finB_hi()
            pop_fillers(1000)
            # lo-half O chunks: PE work while the lo normalizes complete;
            # copies alternate Vector/Scalar so neither engine paces the drain
            for m8 in range(4):
                o_chunk(qi, qs, m8, ow=HW2, oo=0, eng="scalar" if m8 % 2 else None)
            o_flush(qi, qs, 0, 4, co=0, cw=HW2)
            for m8 in range(4, 8):
                o_chunk(qi, qs, m8, ow=HW2, oo=0, eng="scalar" if m8 % 2 else None)
            o_flush(qi, qs, 4, 8, co=0, cw=HW2)
            for m8 in range(4):
                o_chunk(qi, qs, m8, ow=HW2, oo=HW2,
                        eng="scalar" if m8 % 2 else None, alt=(m8 % 2 == 1))
            o_flush(qi, qs, 0, 4, co=HW2, cw=HW2)
            for m8 in range(4, 8):
                o_chunk(qi, qs, m8, ow=HW2, oo=HW2,
                        eng="scalar" if m8 % 2 else None, alt=(m8 % 2 == 1))
            o_flush(qi, qs, 4, 8, co=HW2, cw=HW2)

    nc.compile()
    return nc


_NC_CACHE = {}


def _get_nc(L_c):
    if L_c not in _NC_CACHE:
        _NC_CACHE[L_c] = _build(L_c)
    return _NC_CACHE[L_c]


def _install_ntff_hook():
    """Synthesize antenv.axon_hooks (missing in this image) so trace=True works."""
    import types

    if "antenv.axon_hooks" in sys.modules:
        return
    try:
        if "/root/.axon_site" not in sys.path:
            sys.path.insert(0, "/root/.axon_site")
        from trn_agent_boot.trn_boot import _ntff_profile_via_ctypes

        hook = _ntff_profile_via_ctypes("/opt/axon/libaxon_pjrt.so")
        mod = types.ModuleType("antenv.axon_hooks")
        mod.get_axon_ntff_profile_hook = lambda: hook
        import antenv  # noqa: F401

        sys.modules["antenv.axon_hooks"] = mod
    except Exception:
        pass


def kernel(query, att_mask, Wq, bq, Wk, bk, Wv, bv, Wo, bo):
    global LAST_EXEC_NS, LAST_RESULTS
    query = np.asarray(query, dtype=np.float32)
    mask = np.asarray(att_mask).astype(bool).reshape(B, L)
    Wq, bq = np.asarray(Wq, np.float32), np.asarray(bq, np.float32)
    Wk, bk = np.asarray(Wk, np.float32), np.asarray(bk, np.float32)
    Wv, bv = np.asarray(Wv, np.float32), np.asarray(bv, np.float32)
    Wo, bo = np.asarray(Wo, np.float32), np.asarray(bo, np.float32)

    valid = [np.nonzero(mask[b])[0] for b in range(B)]
    L_c = max(len(v) for v in valid)
    out = np.empty((B, D, L), np.float32)
    if L_c == 0:
        out[:] = bo[None, :, None]
        return out

    scale = np.float32(1.0 / np.sqrt(DK))
    L_c = max(128, L_c)
    mts = _key_tiles(L_c)
    MT = len(mts)
    deadw = MT * 128 - L_c
    xkb = _xk_blocks(L_c)
    # per-batch compacted keys + V-operand init image (zeros, with the
    # ones/denominator column set on live rows only)
    xk_b, vs_b, xb_b = [], [], []
    for b in range(B):
        idx = valid[b]
        xk = np.zeros((D, L_c), np.float32)
        xk[:, :len(idx)] = query[b][:, idx]
        xk8 = xk.astype(NPBF16)
        # partition-major blocks: [128, KT, w] with element [p, t, j] = xk[t*128+p, s+j]
        blocks = [
            np.ascontiguousarray(
                xk8[:, s:s + w].reshape(KT, 128, w).transpose(1, 0, 2)
            )
            for s, w in xkb
        ]
        xk_b.append(blocks)
        vs = np.zeros((128, MT, HPC, 65), np.float32)
        for t, (ms, mw) in enumerate(mts):
            live = (ms + np.arange(128)) < len(idx)
            if t == MT - 1:
                live &= np.arange(128) >= deadw
            vs[:, t, :, 64] = live[:, None].astype(np.float32)
        vs_b.append(np.ascontiguousarray(vs.astype(NPBF16)))
        xbq = query[b].astype(NPBF16)   # [D, L]
        # [p, c, t, j] = xb[t*128+p, c*256+j]
        xb_b.append(np.ascontiguousarray(
            xbq.reshape(KT, 128, 2 * NQB, 256).transpose(1, 2, 0, 3)
        ))

    def wsplit(WT):   # WT [D, DH] -> [128, 2, KT, 128]: [p,h,t,j]=WT[t*128+p, h*128+j]
        return np.ascontiguousarray(
            WT.reshape(KT, 128, 2, 128).transpose(1, 2, 0, 3)
        )

    in_maps = []
    for c in range(NCORES):
        b, g = divmod(c, NCORES // B)
        sl = slice(g * DH, (g + 1) * DH)
        wqT = np.ascontiguousarray((Wq[sl, :] * scale).T).astype(NPBF16)
        wkT = np.ascontiguousarray(Wk[sl, :].T).astype(NPBF16)
        wvT = np.ascontiguousarray(Wv[sl, :].T).astype(NPBF16)
        woT = np.ascontiguousarray(Wo[:, sl].T).astype(NPBF16)
        m = {
            "xb": xb_b[b],
            "vsetup": vs_b[b],
            "wq": wsplit(wqT),
            "wk": wsplit(wkT),
            "wv": np.ascontiguousarray(wvT.reshape(KT, 128, DH).transpose(1, 0, 2)),
            "wo": np.ascontiguousarray(woT.reshape(2, 128, D).transpose(1, 0, 2)),
            "bias": np.stack(
                [(bq[sl] * scale), bk[sl]], axis=-1
            ).reshape(2, 128, 2).transpose(1, 0, 2).reshape(128, 4).astype(np.float32),
        }
        for i in range(len(xkb)):
            m[f"xk{i}"] = xk_b[b][i]
        in_maps.append(m)

    nc = _get_nc(L_c)
    if TRACE:
        _install_ntff_hook()
    res = run_bass_kernel_spmd(nc, in_maps, core_ids=list(range(NCORES)), trace=TRACE)
    LAST_EXEC_NS = res.exec_time_ns
    LAST_RESULTS = res

    bo_eff = (Wo @ bv + bo)[:, None]   # bv folded through the O projection
    parts = [
        res.results[c]["out"].transpose(2, 1, 0, 3).reshape(D, L)
        for c in range(NCORES)
    ]
    for b in range(B):
        if len(valid[b]) == 0:
            out[b] = bo[:, None]
        else:
            acc = parts[4 * b].astype(np.float32)
            for g in range(1, 4):
                acc = acc + parts[4 * b + g]
            out[b] = acc + bo_eff
    return out


# revision 54
# speedup vs baseline: 1.1006x; 1.1006x over previous
"""Trainium2 Bass kernel for nn_MultiHeadAttention_38611755991513.

Reference computation (B=2, D=1024, L=2048, H=16, DK=64):
    q/k/v = conv1d(kernel=1) projections of query [B, D, L]
    att   = softmax(mask(q^T k / sqrt(DK)))   with key-only mask [B, 1, L]
    out   = Wo @ (att @ v heads recombined) + bo

Sharding: 32 (batch, head) pairs -> 4 heads (one batch) per core.
Each core computes its 4 heads' attention plus the partial O-projection
(Wo columns for its heads); the host sums the 4 partials per batch.

Key optimizations:
- Key-only mask -> masked keys compacted away on the host; the ragged last
  128-key tile overlaps the previous one so every tile is full width (the
  duplicated rows are dead: zeroed V rows and ones-column).
- Scores are computed transposed (S^T[k, q]) so exp(S^T) is directly the
  moving operand of att@v; the softmax denominator comes free as a 65th
  "ones" column of the V operand.
- Per (head-pair, key-tile) the two heads' scores land in ONE [128, 2, 512]
  PSUM tile (the K=64 matmuls target row groups 0-63 / 64-127 and stream
  concurrently when issued back-to-back) and are consumed by ONE merged
  exp, which frees both heads' slots simultaneously.
- All DRAM parameters are host-swizzled so both the DRAM source AND the
  SBUF destination of every input DMA are contiguous per-partition runs
  (wq/wk head-pair-major, xb 256-column-chunk-major with 3D moving APs,
  out query-chunk-major): the DMA element size is min(src_run, dst_run),
  and strided destinations were a hidden 256-byte-element bottleneck.
- All input DMAs stay on the single Sync queue in consumption order (bulk
  last): the Tile scheduler models DMA serially per queue, so one queue
  keeps its ETAs accurate and its static schedule matched to reality.
- Phase 0 uses precise per-tile filler deadlines (each k chunk emitted just
  before the first score tile reading it, v chains after their tile's
  scores) plus tile_wait_until hints on late-xk chains, so the early score/
  exp stream is never queued behind fillers stalled on in-flight DMAs and
  the HAM clock stays at 2.4 GHz from ~17us on.
- Tail: the final phase's att@v runs as two 256-wide halves accumulating in
  disjoint column regions of ONE psum tile per head (second half starts
  without re-clearing the bank, so no WAR wait on the first half's
  normalize); the first half's normalize + O chunks overlap the second
  half's matmuls; the last normalizes use a PE broadcast; final O-chunk
  copies alternate Vector/Scalar and rotate through 4 PSUM banks; the
  output flushes are split into quarters so the last DMA is small.
- bv is folded into bo on the host (out = Wo@(y/den) + (Wo@bv + bo)), and
  bq is pre-scaled by 1/sqrt(DK).
"""

import sys

sys.path.insert(0, "/opt/trn_rl_repo")

import numpy as np
import ml_dtypes

import concourse.bass as bass
import concourse.tile as tile
from concourse import bacc, mybir
from concourse.bass_utils import run_bass_kernel_spmd

B, D, L, H = 2, 1024, 2048, 16
DK = 64
NCORES = 8
HPC = 4              # heads per core
DH = HPC * DK        # 256 head-dims per core
KT = D // 128        # 8 contraction tiles for the projections
BF16 = mybir.dt.bfloat16
F32 = mybir.dt.float32
NPBF16 = ml_dtypes.bfloat16

TRACE = False            # set True (e.g. from test.py) to capture a HW profile
LAST_EXEC_NS = None
LAST_RESULTS = None

QBW = 512                # query-block width (one PSUM bank per head slot)
NQB = L // QBW


def _chunks(total, size):
    out = []
    s = 0
    while s < total:
        w = min(size, total - s)
        out.append((s, w))
        s += w
    return out


def _key_tiles(L_c):
    """Full-width 128-key tiles covering [0, L_c); the last tile overlaps the
    previous one when L_c is ragged (its first MT*128-L_c rows are dead)."""
    MT = (L_c + 127) // 128
    mts = [(i * 128, 128) for i in range(MT - 1)]
    mts.append((L_c - 128, 128))
    return mts


def _xk_blocks(L_c):
    """Column blocks of the compacted keys, matching k-chain consumption."""
    if L_c <= 128:
        return [(0, L_c)]
    if L_c <= 512:
        return [(0, 128), (128, L_c - 128)]
    if L_c <= 768:
        return [(0, 128), (128, 384), (512, L_c - 512)]
    return [(0, 128), (128, 384), (512, 256), (768, L_c - 768)]


def _build(L_c):
    """Build + compile the per-core Bass program for compacted key length L_c."""
    assert L_c >= 128
    nc = bacc.Bacc("TRN2", debug=False, num_devices=NCORES)
    mts = _key_tiles(L_c)
    MT = len(mts)
    deadw = MT * 128 - L_c   # dead leading rows of the (overlapped) tail tile
    EXP = mybir.ActivationFunctionType.Exp
    xkb = _xk_blocks(L_c)

    xb_d = nc.declare_dram_parameter("xb", [128, 2 * NQB, KT, 256], BF16, isOutput=False)
    xk_ds = [
        nc.declare_dram_parameter(f"xk{i}", [128, KT, w], BF16, isOutput=False)
        for i, (s, w) in enumerate(xkb)
    ]
    vs_d = nc.declare_dram_parameter("vsetup", [128, MT, HPC, 65], BF16, isOutput=False)
    wq_d = nc.declare_dram_parameter("wq", [128, 2, KT, 128], BF16, isOutput=False)
    wk_d = nc.declare_dram_parameter("wk", [128, 2, KT, 128], BF16, isOutput=False)
    wv_d = nc.declare_dram_parameter("wv", [128, KT, DH], BF16, isOutput=False)
    wo_d = nc.declare_dram_parameter("wo", [128, 2, D], BF16, isOutput=False)
    bias_d = nc.declare_dram_parameter("bias", [128, 4], F32, isOutput=False)
    out_d = nc.declare_dram_parameter("out", [NQB, 128, KT, QBW], BF16, isOutput=True)

    from contextlib import ExitStack
    with tile.TileContext(nc) as tc, ExitStack() as ctx:
        pers = ctx.enter_context(tc.tile_pool(name="pers", bufs=1))

        def ptile(shape, dtype, name):
            return pers.tile(shape, dtype, tag=name, name=name)

        # persistent SBUF tensors.  wq/wk are head-pair-major and xb is
        # 256-column-chunk-major so each input DMA writes one fully
        # CONTIGUOUS per-partition run (the element size a DMA moves is
        # min(src_run, dst_run) -- strided SBUF destinations were the
        # hidden 256-byte-element bottleneck).
        xk_a = ptile([128, KT, L_c], BF16, "xk_a")
        xb_a = ptile([128, 2 * NQB, KT, 256], BF16, "xb_a")
        wq_a = ptile([128, 2, KT, 128], BF16, "wq_a")
        wk_a = ptile([128, 2, KT, 128], BF16, "wk_a")
        wv_a = ptile([128, KT, DH], BF16, "wv_a")
        wo_a = ptile([128, 2, D], BF16, "wo_a")
        xk_t = [xk_a[:, i] for i in range(KT)]
        wv_t = [wv_a[:, i] for i in range(KT)]
        wo_t = [wo_a[:, i] for i in range(2)]

        def xb_sl(kk, qs, jw):
            """Moving-operand view of xb columns [qs, qs+jw) for D-tile kk."""
            c0, nch = qs // 256, jw // 256
            if nch == 1:
                return xb_a[:, c0, kk]
            return xb_a[:, c0:c0 + nch, kk]   # [128, nch, 256] free=jw
        bias_all = ptile([128, 4], F32, "bias_all")
        bq_t = [bias_all[:, 2 * i + 0:2 * i + 1] for i in range(2)]
        bk_t = [bias_all[:, 2 * i + 1:2 * i + 2] for i in range(2)]
        q_t = [ptile([128, L], BF16, f"q{i}") for i in range(2)]
        k_t = [ptile([128, L_c], BF16, f"k{i}") for i in range(2)]
        z_t = [ptile([128, L], BF16, f"z{i}") for i in range(2)]
        # V operand per key tile: [128, head, 65]; col 64 is the ones column
        # (denominator); vsetup pre-zeroes dead rows and sets the ones
        vs_a = ptile([128, MT, HPC, 65], BF16, "vs_a")
        va_t = [vs_a[:, mt] for mt in range(MT)]
        # per-qblock output staging, so each qblock stores with ONE DMA
        ob_a = [ptile([128, 8, QBW], BF16, f"ob{i}") for i in range(2)]
        ones_t = ptile([1, 64], F32, "ones_t")
        # ---- input DMAs: three queues (sync/scalar HWDGE + gpsimd SWDGE)
        # issue in parallel, each in consumption order, with the bulk last:
        # in-flight DMAs share the ~360 GB/s fabric at packet granularity,
        # so anything enqueued early steals bandwidth from the critical
        # first-wave slices.
        # NOTE: scalar (ACT) also has an HWDGE ring but issuing DMAs there
        # delays the activation-table load and the first exp -- keep it clean.
        # single queue by default: the Tile scheduler models DMA transfers
        # serially per queue, and with one queue its ETAs match reality --
        # split queues made it schedule consumers ahead of slow transfers.
        import os
        if os.environ.get("MULTI_QUEUE"):
            q_xk, q_rest = nc.gpsimd, nc.sync
        else:
            q_xk = q_rest = nc.sync
        # critical wave: first k chain needs xk[:,0:128]+wk half 0; the
        # prologue q chains need wq half 0 + xb cols 0:512; biases gate the
        # first PSUM copy-outs.
        # sync carries only the small critical set; the xk blocks AND the
        # late-consumed bulk (xb blocks 2-7, wo) queue serially on gpsimd so
        # the bulk cannot steal fabric bandwidth from xk during the ramp
        # (phase 0 consumes xk progressively and is transfer-bound).
        q_xk.dma_start(xk_a[:, :, xkb[0][0]:xkb[0][0] + xkb[0][1]], xk_ds[0][:])
        q_rest.dma_start(wk_a[:, 0], wk_d[:, 0])
        q_rest.dma_start(bias_all[:], bias_d[:])
        q_rest.dma_start(wq_a[:, 0], wq_d[:, 0])
        q_rest.dma_start(xb_a[:, 0:2], xb_d[:, 0:2])
        for i in range(1, len(xkb)):
            s, w = xkb[i]
            q_xk.dma_start(xk_a[:, :, s:s + w], xk_ds[i][:])
        q_rest.dma_start(wk_a[:, 1], wk_d[:, 1])
        q_rest.dma_start(wv_a[:], wv_d[:])
        q_rest.dma_start(wq_a[:, 1], wq_d[:, 1])
        q_xk.dma_start(vs_a[:], vs_d[:])
        q_xk.dma_start(xb_a[:, 2:4], xb_d[:, 2:4])
        q_xk.dma_start(xb_a[:, 4:6], xb_d[:, 4:6])
        q_xk.dma_start(wo_a[:], wo_d[:])
        q_xk.dma_start(xb_a[:, 6:8], xb_d[:, 6:8])

        with (
            tc.tile_pool(name="psA", bufs=2, space="PSUM") as pa,
            tc.tile_pool(name="psY", bufs=2, space="PSUM") as pb,
            tc.tile_pool(name="psO", bufs=2, space="PSUM") as pox,
            tc.tile_pool(name="pexp", bufs=2 * MT + 4) as pp,
            tc.tile_pool(name="small", bufs=3) as psm,
        ):
            def k_chain(kt, ns, nw, halves=None, wait=None):
                kp = pox.tile([128, nw], F32, tag="po", name=f"kp{kt}_{ns}")

                def half(lo, hi):
                    # `wait` floors the Tile scheduler's assumed ready time:
                    # its DMA model ignores fabric contention, so without it
                    # late-xk chains get scheduled AHEAD of already-ready
                    # scores and the PE stalls on the real transfer.
                    from contextlib import nullcontext
                    with tc.tile_wait_until(wait) if wait else nullcontext():
                        for kk in range(lo, hi):
                            nc.tensor.matmul(
                                kp[:],
                                wk_a[:, kt, kk],
                                xk_t[kk][:, ns:ns + nw],
                                start=(kk == 0), stop=(kk == KT - 1),
                            )
                        if hi == KT:
                            nc.vector.tensor_scalar_add(
                                k_t[kt][:, ns:ns + nw], kp[:], bk_t[kt][:]
                            )
                if halves is None:
                    half(0, KT)
                else:
                    halves.append(lambda: half(0, KT // 2))
                    halves.append(lambda: half(KT // 2, KT))

            def q_chain(qs, kt, halves=None, jw=QBW):
                qp = pox.tile([128, jw], F32, tag="po", name=f"qp{kt}_{qs}")

                def half(lo, hi):
                    for kk in range(lo, hi):
                        nc.tensor.matmul(
                            qp[:],
                            wq_a[:, kt, kk],
                            xb_sl(kk, qs, jw),
                            start=(kk == 0), stop=(kk == KT - 1),
                        )
                    if hi == KT:
                        nc.vector.tensor_scalar_add(q_t[kt][:, qs:qs + jw], qp[:], bq_t[kt][:])
                if halves is None:
                    half(0, KT)
                else:
                    halves.append(lambda: half(0, KT // 2))
                    halves.append(lambda: half(KT // 2, KT))

            def v_chain(mt):
                from contextlib import nullcontext
                ms, mw = mts[mt]
                wait = (0.021 + 0.0012 * (mt - 4)) if ms + mw > 512 else None
                vp = pox.tile([mw, DH], F32, tag="po", name=f"vp{mt}")
                with tc.tile_wait_until(wait) if wait else nullcontext():
                    for kk in range(KT):
                        nc.tensor.matmul(
                            vp[:],
                            xk_t[kk][:, ms:ms + mw],
                            wv_t[kk][:],
                            start=(kk == 0), stop=(kk == KT - 1),
                        )
                    for h in range(HPC):
                        nc.vector.tensor_copy(
                            va_t[mt][:, h, 0:64], vp[:, h * 64:(h + 1) * 64]
                        )
                    if mt == MT - 1 and deadw:
                        # re-zero the dead overlap rows the copy just filled
                        nc.vector.memset(va_t[mt][0:deadw, :, 0:64], 0)

            def o_chunk(qi, qs, m8, ow=QBW, oo=0, eng=None, alt=False):
                # alt: borrow the y pool's (free-at-drain) PSUM ring so the
                # final chunks rotate through 4 banks instead of 2 -- the
                # drain is paced by the copy-out round-trip latency.
                if alt:
                    op = pb.tile([128, ow], F32, tag="y", name=f"o{qs}_{m8}_{oo}")
                else:
                    op = pox.tile([128, ow], F32, tag="po", name=f"o{qs}_{m8}_{oo}")
                for kt in range(2):
                    nc.tensor.matmul(
                        op[:],
                        wo_t[kt][:, m8 * 128:(m8 + 1) * 128],
                        z_t[kt][:, qs + oo:qs + oo + ow],
                        start=(kt == 0), stop=(kt == 1),
                    )
                dst = ob_a[qi % 2][:, m8, oo:oo + ow]
                if eng == "scalar":
                    nc.scalar.copy(dst, op[:])
                else:
                    nc.vector.tensor_copy(dst, op[:])

            def o_flush(qi, qs, lo=0, hi=8, co=0, cw=QBW):
                nc.sync.dma_start(
                    out_d[qi][:, lo:hi, co:co + cw],
                    ob_a[qi % 2][:, lo:hi, co:co + cw],
                )

            # ---- prologue ----
            # DMA-independent warmup on the zeroed dummy tiles: keeps PE busy
            # from ~7us (framework preamble end) so HAM un-throttles by the
            # time real data lands, without delaying the first k chain much.
            if L_c <= 128:
                kchunks = [(0, L_c)]
            elif L_c <= 512:
                kchunks = [(0, 128), (128, L_c - 128)]
            else:
                kchunks = [(0, 128), (128, 384)] + _chunks(L_c, 512)[1:]
            nc.vector.memset(ones_t[:], 1.0)
            k_chain(0, *kchunks[0])
            # HAM warmup on already-loaded data: keeps the PE streaming while
            # the first Q-block DMAs land so the clock ramps to 2.4 GHz
            for w in range(7):
                wu = pox.tile([128, 128], F32, tag="po", name=f"wu{w}")
                for kk in range(KT):
                    nc.tensor.matmul(
                        wu[:],
                        wk_a[:, 0, kk],
                        xk_t[kk][:, 0:128],
                        start=(kk == 0), stop=(kk == KT - 1),
                    )
            q_chain(0, 0, jw=256)
            q_chain(256, 0, jw=256)

            # ---- software-pipelined attention, head-PAIR phases ----
            def y_head(h, qs, p_tiles, yq, yw=QBW, yo=0, pe_bcast=False, share=None,
                       ypool=None):
                # ypool: the drain's second halves allocate from the score
                # pool's (free-by-then) slots instead of rotating pb's two --
                # Tile tracks PSUM deps per TILE, so any sharing or rotation
                # falsely serializes the halves against the normalize reads.
                state = {} if share is None else share

                def y_mt(mt):
                    first_owner = "yp" not in state
                    if mt == 0 and first_owner:
                        if ypool is not None:
                            state["yp"] = ypool.tile([65, QBW], F32, tag="wide",
                                                     name=f"y{qs}_{h}_{yo}")
                        else:
                            state["yp"] = pb.tile([65, QBW], F32, tag="y",
                                                  name=f"y{qs}_{h}_{yo}")
                    # a second half sharing the tile must NOT re-clear the
                    # bank (start=True zeroes the whole bank's has_written):
                    # its region is untouched, so plain accumulate-mode
                    # writes it fresh via the per-element has_written bits.
                    nc.tensor.matmul(
                        state["yp"][:, yo:yo + yw],
                        va_t[mt][:, h, :],
                        p_tiles[mt][:, h % 2, yo:yo + yw],
                        start=(mt == 0 and (share is None or first_owner)),
                        stop=(mt == MT - 1),
                        skip_group_check=(share is not None),
                    )

                def finish():
                    pt, off = h // 2, (h % 2) * 64
                    yp = state["yp"]
                    rt = psm.tile([1, QBW], F32, tag="rrow", name=f"rt{qs}_{h}_{yo}")
                    nc.vector.tensor_copy(rt[:, 0:yw], yp[64:65, yo:yo + yw])
                    rc = psm.tile([1, QBW], F32, tag="recip", name=f"rc{qs}_{h}_{yo}")
                    nc.vector.reciprocal_approx_fast(rc[:, 0:yw], rt[:, 0:yw])
                    zsl = z_t[pt][off:off + 64, qs + yo:qs + yo + yw]
                    if pe_bcast:
                        rbp = pox.tile([128, QBW], F32, tag="po", name=f"rb{qs}_{h}_{yo}")
                        nc.tensor.matmul(
                            rbp[0:64, 0:yw], ones_t[:], rc[:, 0:yw],
                            start=True, stop=True,
                        )
                        rbs = psm.tile([64, QBW], F32, tag="rb", name=f"rs{qs}_{h}_{yo}")
                        nc.vector.tensor_copy(rbs[:, 0:yw], rbp[0:64, 0:yw])
                        nc.vector.tensor_mul(zsl, yp[0:64, yo:yo + yw], rbs[:, 0:yw])
                    else:
                        rb = psm.tile([64, QBW], F32, tag="rb", name=f"rb{qs}_{h}_{yo}")
                        nc.gpsimd.partition_broadcast(rb[:, 0:yw], rc[:, 0:yw])
                        nc.vector.tensor_mul(zsl, yp[0:64, yo:yo + yw], rb[:, 0:yw])

                for mt in range(0, MT, 2):
                    def two(mt=mt):
                        y_mt(mt)
                        if mt + 1 < MT:
                            y_mt(mt + 1)
                    yq.append(two)
                yq.append(finish)
                return finish

            fillers = []   # (cost, emit) pairs
            fi = 0

            def pop_fillers(budget):
                nonlocal fi
                while budget > 0 and fi < len(fillers):
                    cost, emit = fillers[fi]
                    emit()
                    fi += 1
                    budget -= cost
                return budget

            def pop_until(idx):
                nonlocal fi
                while fi < idx:
                    fillers[fi][1]()
                    fi += 1

            # K/Q/V chains are PREREQUISITES of later emissions: Tile derives
            # dependencies from emission order, so a consumer emitted before
            # its writer would silently read stale data.
            def k_wait(ns):
                # approximate real arrival of the xk columns feeding [ns, ...)
                if ns < 384:
                    return None
                return 0.019 + 0.000009 * ns

            halves = []
            k0_dead_raw = []   # (first tile needing this chunk, filler idx after it)
            for ns, nw in kchunks[1:]:
                k_chain(0, ns, nw, halves, wait=k_wait(ns))
                t_first = next(t for t, (ms, mw) in enumerate(mts) if ms + mw > ns)
                k0_dead_raw.append((t_first, len(halves)))
            k0_end = len(halves)
            for ns, nw in kchunks:
                k_chain(1, ns, nw, halves, wait=k_wait(ns))
            fillers.extend((4, fn) for fn in halves)
            vk_deadline = len(fillers)
            # phase-0 per-tile deadlines: each k(0) chunk is emitted just
            # before the first tile whose scores read it (so early tiles'
            # scores aren't queued behind fillers waiting on late xk DMA),
            # and the k(1)/q(0,1) chains spread over the last 4 tiles.
            K0_DEAD = [0] * MT
            for t_first, idx in k0_dead_raw:
                for t in range(t_first, MT):
                    K0_DEAD[t] = max(K0_DEAD[t], idx)
            for j, t in enumerate(range(max(0, MT - 4), MT)):
                frac = (j + 1) / min(4, MT)
                tgt = k0_end + int(frac * (vk_deadline + 2 - k0_end))
                K0_DEAD[t] = max(K0_DEAD[t], tgt)
            deadline = {}
            after_block = {}
            for qi in range(NQB):
                for hp in range(2):
                    if (qi, hp) == (0, 0):
                        continue   # prologue chains
                    halves = []
                    q_chain(qi * QBW, hp, halves)
                    fillers.extend((4, fn) for fn in halves)
                after_block[qi] = len(fillers)
            # pop each block's q chains ONE PHASE EARLY: their bias copies
            # (DVE) then land before the phase that reads them starts, so the
            # phase's first score pair doesn't stall on the copy latency.
            for qi in range(NQB):
                nxt = min(qi + 1, NQB - 1)
                deadline[(qi, 0)] = deadline[(qi, 1)] = after_block[nxt]

            qblocks = _chunks(L, QBW)
            yq = []       # pending y work units of the previous pair

            for qi, (qs, qw) in enumerate(qblocks):
                for hp in range(2):
                    hA, hB = 2 * hp, 2 * hp + 1
                    first_phase = (qi, hp) == (0, 0)
                    last_phase = (qi, hp) == (len(qblocks) - 1, 1)
                    pop_until(deadline.get((qi, hp), 0))   # q chains this phase reads
                    if hp == 1 and qi >= 1:
                        # z of block qi-1 completed during the previous phase:
                        # its O-projection chunks become filler work now.
                        pqs = qblocks[qi - 1][0]
                        for m8 in range(8):
                            fillers.append(
                                (4, lambda qi=qi, pqs=pqs, m8=m8: o_chunk(qi - 1, pqs, m8))
                            )
                        fillers.append((0, lambda qi=qi, pqs=pqs: o_flush(qi - 1, pqs)))
                    ptiles = []
                    for mt, (ms, mw) in enumerate(mts):
                        for _ in range(2):
                            if yq:
                                yq.pop(0)()
                        if first_phase:
                            pop_until(min(K0_DEAD[mt], vk_deadline + 2))
                        else:
                            pop_fillers(4)
                        sp = pa.tile([128, 2, QBW], F32, tag="wide", name=f"s{qs}_{hp}_{mt}")
                        for sl, off in ((0, 0), (1, 64)):
                            nc.tensor.matmul(
                                sp[:, sl, :],
                                k_t[hp][off:off + 64, ms:ms + mw],
                                q_t[hp][off:off + 64, qs:qs + QBW],
                                start=True, stop=True,
                            )
                        px = pp.tile([128, 2, QBW], BF16, tag="p", name=f"p{qs}_{hp}_{mt}")
                        nc.scalar.activation(px[:], sp[:], EXP)
                        ptiles.append(px)
                        if first_phase:
                            # after the tile's scores: the v chain only feeds
                            # the y units at phase end, and wv/vs land late
                            v_chain(mt)
                    while yq:
                        yq.pop(0)()
                    yq = []
                    if first_phase:
                        pop_until(vk_deadline)   # v_chains feed the y units below
                    if not last_phase:
                        y_head(hA, qs, ptiles, yq)
                        y_head(hB, qs, ptiles, yq)
                    else:
                        last_ptiles = ptiles

            # ---- drain (last phase = (NQB-1, 1)) ----
            # The final pair's att@v runs in two 256-wide halves so the first
            # half's normalize + O chunks overlap the second half's matmuls.
            qi = len(qblocks) - 1
            qs = qblocks[-1][0]
            hA, hB = 2, 3
            HW2 = QBW // 2
            ylo = []
            finA_lo = y_head(hA, qs, last_ptiles, ylo, yw=HW2, yo=0)
            finB_lo = y_head(hB, qs, last_ptiles, ylo, yw=HW2, yo=0)
            # emit lo y matmuls now (finishes held), interleaving pairs
            for fn in ylo:
                if fn not in (finA_lo, finB_lo):
                    fn()
            yhi = []
            finA_hi = y_head(hA, qs, last_ptiles, yhi, yw=HW2, yo=HW2,
                             pe_bcast=True, ypool=pa)
            finB_hi = y_head(hB, qs, last_ptiles, yhi, yw=HW2, yo=HW2,
                             pe_bcast=True, ypool=pa)
            for fn in yhi:
                if fn not in (finA_hi, finB_hi):
                    fn()
            finA_lo()   # gpsimd-broadcast path, overlaps the hi matmuls
            finB_lo()
            finA_hi()
            finB_hi()
            pop_fillers(1000)
            # final O chunks at FULL width once z-hi lands: half the units of
            # a 256-wide split, so the (copy-latency-paced) drain is shorter
            # than starting earlier on narrow chunks.  Copies alternate
            # Vector/Scalar and the PSUM rotates through pox+pb (4 banks).
            for m8 in range(4):
                o_chunk(qi, qs, m8, eng="scalar" if m8 % 2 else None,
                        alt=(m8 % 2 == 1))
            o_flush(qi, qs, 0, 4)
            for m8 in range(4, 8):
                o_chunk(qi, qs, m8, eng="scalar" if m8 % 2 else None,
                        alt=(m8 % 2 == 1))
            o_flush(qi, qs, 4, 8)

    nc.compile()
    return nc


_NC_CACHE = {}


def _get_nc(L_c):
    if L_c not in _NC_CACHE:
        _NC_CACHE[L_c] = _build(L_c)
    return _NC_CACHE[L_c]


def _install_ntff_hook():
    """Synthesize antenv.axon_hooks (missing in this image) so trace=True works."""
    import types

    if "antenv.axon_hooks" in sys.modules:
        return
    try:
        if "/root/.axon_site" not in sys.path:
            sys.path.insert(0, "/root/.axon_site")
        from trn_agent_boot.trn_boot import _ntff_profile_via_ctypes

        hook = _ntff_profile_via_ctypes("/opt/axon/libaxon_pjrt.so")
        mod = types.ModuleType("antenv.axon_hooks")
        mod.get_axon_ntff_profile_hook = lambda: hook
        import antenv  # noqa: F401

        sys.modules["antenv.axon_hooks"] = mod
    except Exception:
        pass


def kernel(query, att_mask, Wq, bq, Wk, bk, Wv, bv, Wo, bo):
    global LAST_EXEC_NS, LAST_RESULTS
    query = np.asarray(query, dtype=np.float32)
    mask = np.asarray(att_mask).astype(bool).reshape(B, L)
    Wq, bq = np.asarray(Wq, np.float32), np.asarray(bq, np.float32)
    Wk, bk = np.asarray(Wk, np.float32), np.asarray(bk, np.float32)
    Wv, bv = np.asarray(Wv, np.float32), np.asarray(bv, np.float32)
    Wo, bo = np.asarray(Wo, np.float32), np.asarray(bo, np.float32)

    valid = [np.nonzero(mask[b])[0] for b in range(B)]
    L_c = max(len(v) for v in valid)
    out = np.empty((B, D, L), np.float32)
    if L_c == 0:
        out[:] = bo[None, :, None]
        return out

    scale = np.float32(1.0 / np.sqrt(DK))
    L_c = max(128, L_c)
    mts = _key_tiles(L_c)
    MT = len(mts)
    deadw = MT * 128 - L_c
    xkb = _xk_blocks(L_c)
    # per-batch compacted keys + V-operand init image (zeros, with the
    # ones/denominator column set on live rows only)
    xk_b, vs_b, xb_b = [], [], []
    for b in range(B):
        idx = valid[b]
        xk = np.zeros((D, L_c), np.float32)
        xk[:, :len(idx)] = query[b][:, idx]
        xk8 = xk.astype(NPBF16)
        # partition-major blocks: [128, KT, w] with element [p, t, j] = xk[t*128+p, s+j]
        blocks = [
            np.ascontiguousarray(
                xk8[:, s:s + w].reshape(KT, 128, w).transpose(1, 0, 2)
            )
            for s, w in xkb
        ]
        xk_b.append(blocks)
        vs = np.zeros((128, MT, HPC, 65), np.float32)
        for t, (ms, mw) in enumerate(mts):
            live = (ms + np.arange(128)) < len(idx)
            if t == MT - 1:
                live &= np.arange(128) >= deadw
            vs[:, t, :, 64] = live[:, None].astype(np.float32)
        vs_b.append(np.ascontiguousarray(vs.astype(NPBF16)))
        xbq = query[b].astype(NPBF16)   # [D, L]
        # [p, c, t, j] = xb[t*128+p, c*256+j]
        xb_b.append(np.ascontiguousarray(
            xbq.reshape(KT, 128, 2 * NQB, 256).transpose(1, 2, 0, 3)
        ))

    def wsplit(WT):   # WT [D, DH] -> [128, 2, KT, 128]: [p,h,t,j]=WT[t*128+p, h*128+j]
        return np.ascontiguousarray(
            WT.reshape(KT, 128, 2, 128).transpose(1, 2, 0, 3)
        )

    in_maps = []
    for c in range(NCORES):
        b, g = divmod(c, NCORES // B)
        sl = slice(g * DH, (g + 1) * DH)
        wqT = np.ascontiguousarray((Wq[sl, :] * scale).T).astype(NPBF16)
        wkT = np.ascontiguousarray(Wk[sl, :].T).astype(NPBF16)
        wvT = np.ascontiguousarray(Wv[sl, :].T).astype(NPBF16)
        woT = np.ascontiguousarray(Wo[:, sl].T).astype(NPBF16)
        m = {
            "xb": xb_b[b],
            "vsetup": vs_b[b],
            "wq": wsplit(wqT),
            "wk": wsplit(wkT),
            "wv": np.ascontiguousarray(wvT.reshape(KT, 128, DH).transpose(1, 0, 2)),
            "wo": np.ascontiguousarray(woT.reshape(2, 128, D).transpose(1, 0, 2)),
            "bias": np.stack(
                [(bq[sl] * scale), bk[sl]], axis=-1
            ).reshape(2, 128, 2).transpose(1, 0, 2).reshape(128, 4).astype(np.float32),
        }
        for i in range(len(xkb)):
            m[f"xk{i}"] = xk_b[b][i]
        in_maps.append(m)

    nc = _get_nc(L_c)
    if TRACE:
        _install_ntff_hook()
    res = run_bass_kernel_spmd(nc, in_maps, core_ids=list(range(NCORES)), trace=TRACE)
    LAST_EXEC_NS = res.exec_time_ns
    LAST_RESULTS = res

    bo_eff = (Wo @ bv + bo)[:, None]   # bv folded through the O projection
    parts = [
        res.results[c]["out"].transpose(2, 1, 0, 3).reshape(D, L)
        for c in range(NCORES)
    ]
    for b in range(B):
        if len(valid[b]) == 0:
            out[b] = bo[:, None]
        else:
            acc = parts[4 * b].astype(np.float32)
            for g in range(1, 4):
                acc = acc + parts[4 * b + g]
            out[b] = acc + bo_eff
    return out
